# revision 1
# baseline (speedup 1.0000x reference)
"""Bass/Tile kernel builder for sparse sliding-window attention with sinks.

Problem (full): B=4, N=1024, DIM=1024, H=16, D=64, SW=256.
Sharding: 8 cores; core c -> batch b=c//2, head-group g=c%2 (8 heads each).
Host sums the two per-head-group partial projections + proj bias.

Per-core pipeline:
  A) QKV matmul (f32r) in [n,f] layout -> LN (stats via ACT accum_out,
     apply via DVE tensor_scalar) -> RoPE (cos/sin tables with qk-norm
     weights folded on host) -> PE-transpose q,k to [d,n] layout.
  B) per (head, 128-query block): scores = qT.T @ kT over <=3 key blocks
     (sliding window), additive -1e30 masks on boundary blocks, softmax
     without max-subtraction (logits bounded by |q||k|/8 <= 8), denom via
     exp accum_out + host-precomputed exp(sink), p normalized by 1/denom,
     PE-transpose p (bf16), PV matmul (bf16) accumulating attn^T [d, n].
  C) proj: y[n,e] = attn^T.T @ projT (f32r), DMA psum -> DRAM.
"""

import sys

sys.path.insert(0, "/opt/trn_rl_repo")

import numpy as np
import ml_dtypes

import concourse.bass as bass
import concourse.mybir as mybir
import concourse.tile as tile
from concourse import bacc

F32 = mybir.dt.float32
F32R = mybir.dt.float32r
BF16 = mybir.dt.bfloat16

B, N, DIM = 4, 1024, 1024
H, D = 16, 64
SW = 256
ROPE_BASE = 10000.0
LN_EPS = 1e-5
P = 128
NT = N // P      # 8 query/n tiles
CC = DIM // P    # 8 contraction chunks
HL = H // 2      # 8 local heads
NEG = -1.0e30


def r32(ap):
    return ap.bitcast(F32R)


def build_nc(repeat=1, use_for_i=False, phases="ABC"):
    """Build the per-core Bass graph. repeat>1 replicates the whole body
    (static or via For_i) for wall-clock timing."""
    nc = bacc.Bacc("TRN2", target_bir_lowering=False, debug=False, num_devices=8)

    xt = nc.declare_dram_parameter("xt", [DIM, N], BF16, isOutput=False)
    wqkt = nc.declare_dram_parameter("wqkt", [DIM, 1024], BF16, isOutput=False)
    wvt = nc.declare_dram_parameter("wvt", [DIM, 512], BF16, isOutput=False)
    projt = nc.declare_dram_parameter("projt", [512, DIM], F32, isOutput=False)
    coswq = nc.declare_dram_parameter("coswq", [N, D], F32, isOutput=False)
    sinwq = nc.declare_dram_parameter("sinwq", [N, D], F32, isOutput=False)
    coswk = nc.declare_dram_parameter("coswk", [N, D], F32, isOutput=False)
    sinwk = nc.declare_dram_parameter("sinwk", [N, D], F32, isOutput=False)
    esink = nc.declare_dram_parameter("esink", [1, HL], F32, isOutput=False)
    masks01 = nc.declare_dram_parameter("masks01", [P, 2 * P], BF16, isOutput=False)
    identf = nc.declare_dram_parameter("identf", [P, P], F32, isOutput=False)
    identb = nc.declare_dram_parameter("identb", [P, P], BF16, isOutput=False)
    y = nc.declare_dram_parameter("y", [N, DIM], F32, isOutput=True)

    with tile.TileContext(nc) as tc:
        with tc.tile_pool(name="consts", bufs=1) as consts:
            wqk_sb = consts.tile([P, CC, 1024], BF16, tag="wqk")
            wqk_src = wqkt.ap().rearrange("(cc p) f -> p cc f", p=P)
            wv_sb = consts.tile([P, CC, 512], BF16, tag="wv")
            wv_src = wvt.ap().rearrange("(cc p) f -> p cc f", p=P)
            for c in range(CC):
                nc.sync.dma_start(out=wqk_sb[:, c, :], in_=wqk_src[:, c, :])
                nc.sync.dma_start(out=wv_sb[:, c, :], in_=wv_src[:, c, :])
            pj_sb = consts.tile([P, 4, DIM], F32R, tag="pj")
            pj_src = projt.ap().rearrange("(ch p) e -> p ch e", p=P).bitcast(F32R)
            for ch in range(4):
                nc.sync.dma_start(out=pj_sb[:, ch, :], in_=pj_src[:, ch, :])
            cos_sin = {}
            for nm, t_dram in (
                ("cq", coswq), ("sq", sinwq), ("ck", coswk), ("sk", sinwk),
            ):
                t_sb = consts.tile([P, NT, D], F32, tag=nm)
                nc.sync.dma_start(
                    out=t_sb, in_=t_dram.ap().rearrange("(t p) d -> p t d", p=P))
                cos_sin[nm] = t_sb
            es_sb = consts.tile([P, HL], F32, tag="es")
            nc.sync.dma_start(out=es_sb, in_=esink.ap().to_broadcast([P, HL]))
            mk_sb = consts.tile([P, 2 * P], BF16, tag="mk")
            nc.sync.dma_start(out=mk_sb, in_=masks01.ap())
            idf_sb = consts.tile([P, P], F32, tag="idf")
            nc.sync.dma_start(out=idf_sb, in_=identf.ap())
            idb_sb = consts.tile([P, P], BF16, tag="idb")
            nc.sync.dma_start(out=idb_sb, in_=identb.ap())
            eps_sb = consts.tile([P, 1], F32, tag="eps")
            nc.vector.memset(eps_sb, LN_EPS)

            # persistent intermediates
            qkt_sb = consts.tile([P, 8, N], F32R, tag="qkt")  # [f, pair(q0-3,k4-7), n]
            v_sb = consts.tile([P, NT, 512], BF16, tag="v")   # [n, ktile, hd]
            att_sb = consts.tile([P, 4, NT, P], F32R, tag="att")  # [hd, chunk, nt, n]
            zraw = consts.tile([P, NT, 1024], F32, tag="zraw")   # raw qk pre-LN
            st_g = consts.tile([P, NT, 16], F32, tag="stg")      # sums
            sq_g = consts.tile([P, NT, 16], F32, tag="sqg")      # sumsq
            rstd_g = consts.tile([P, NT, 16], F32, tag="rstdg")
            mrs_g = consts.tile([P, NT, 16], F32, tag="mrsg")

            def body(phases=phases):
                with (
                    tc.tile_pool(name="pA", bufs=3) as pA,
                    tc.tile_pool(name="psA", bufs=2, space="PSUM") as psA,
                    tc.tile_pool(name="pR", bufs=1) as pR,
                ):
                    do_ln = ('L' in phases) or ('A' in phases)

                    def a1_tile(t):
                        nsl = slice(t * P, (t + 1) * P)
                        xt_t = pA.tile([P, CC, P], BF16, tag="xt", name=f"xt{t}")
                        nc.sync.dma_start(
                            out=xt_t,
                            in_=xt[:, nsl].rearrange("(cc p) n -> p cc n", p=P))
                        ps_q = psA.tile([P, 512], F32, tag="psq", name=f"psq{t}")
                        ps_k = psA.tile([P, 512], F32, tag="psk", name=f"psk{t}")
                        ps_v = psA.tile([P, 512], F32, tag="psv", name=f"psv{t}")
                        for c in range(CC):
                            st, sp = (c == 0), (c == CC - 1)
                            nc.tensor.matmul(ps_q, xt_t[:, c, :], wqk_sb[:, c, 0:512],
                                             start=st, stop=sp)
                            nc.tensor.matmul(ps_k, xt_t[:, c, :], wqk_sb[:, c, 512:1024],
                                             start=st, stop=sp)
                            nc.tensor.matmul(ps_v, xt_t[:, c, :], wv_sb[:, c, :],
                                             start=st, stop=sp)
                        nc.scalar.copy(out=v_sb[:, t, :], in_=ps_v)
                        if not do_ln:
                            return
                        scr = pA.tile([P, 512], F32, tag="scr", name=f"scr{t}")
                        for half, ps_h in enumerate((ps_q, ps_k)):
                            hsl = slice(half * 8, half * 8 + 8)
                            ps3 = ps_h.rearrange("p (h d) -> p h d", d=D)
                            nc.vector.tensor_reduce(
                                out=st_g[:, t, hsl], in_=ps3,
                                axis=mybir.AxisListType.X, op=mybir.AluOpType.add)
                            nc.scalar.activation(
                                out=scr, in_=ps_h,
                                func=mybir.ActivationFunctionType.Square)
                            nc.vector.tensor_reduce(
                                out=sq_g[:, t, hsl],
                                in_=scr.rearrange("p (h d) -> p h d", d=D),
                                axis=mybir.AxisListType.X, op=mybir.AluOpType.add)
                            nc.scalar.copy(
                                out=zraw[:, t, half * 512:(half + 1) * 512],
                                in_=ps_h)

                    def stats_pair(g):
                        tsl = slice(g * 2, g * 2 + 2)
                        mean_g = pR.tile([P, 2, 16], F32, tag=f"meang{g % 2}",
                                         name=f"meang{g}")
                        var_g = pR.tile([P, 2, 16], F32, tag=f"varg{g % 2}",
                                        name=f"varg{g}")
                        nc.scalar.mul(out=mean_g, in_=st_g[:, tsl, :], mul=1.0 / D)
                        nc.vector.scalar_tensor_tensor(
                            out=var_g, in0=st_g[:, tsl, :], scalar=1.0, in1=mean_g,
                            op0=mybir.AluOpType.mult, op1=mybir.AluOpType.mult)
                        nc.vector.tensor_sub(out=var_g, in0=sq_g[:, tsl, :], in1=var_g)
                        nc.scalar.activation(
                            out=var_g, in_=var_g, scale=1.0 / D,
                            func=mybir.ActivationFunctionType.Sqrt, bias=eps_sb)
                        nc.vector.reciprocal(out=rstd_g[:, tsl, :], in_=var_g)
                        nc.vector.tensor_mul(out=mrs_g[:, tsl, :], in0=mean_g,
                                             in1=rstd_g[:, tsl, :])

                    def a3_tiles(ts):
                        rots, tmps = {}, {}
                        for t in ts:
                            rots[t] = pR.tile([P, 1024], F32, tag=f"rot{t % 4}",
                                              name=f"rotb{t}")
                            tmps[t] = pR.tile([P, 1024], F32, tag=f"tmp{t % 4}",
                                              name=f"tmpb{t}")
                        for t in ts:
                            z16 = zraw[:, t, :].rearrange("p (s d) -> p s d", d=D)
                            rb = rstd_g[:, t, :].unsqueeze(2).broadcast_to([P, 16, D])
                            mb = mrs_g[:, t, :].unsqueeze(2).broadcast_to([P, 16, D])
                            nc.vector.tensor_mul(out=z16, in0=z16, in1=rb)
                            nc.vector.tensor_sub(out=z16, in0=z16, in1=mb)
                        for t in ts:
                            zv = zraw[:, t, :].rearrange(
                                "p (s h j) -> p s h j", h=2, j=D // 2)
                            rv = rots[t].rearrange("p (s h j) -> p s h j",
                                                   h=2, j=D // 2)
                            nc.gpsimd.tensor_copy(out=rv[:, :, 0, :],
                                                  in_=zv[:, :, 1, :])
                            nc.gpsimd.tensor_copy(out=rv[:, :, 1, :],
                                                  in_=zv[:, :, 0, :])
                        for t in ts:
                            for half, (cn, sn) in enumerate(
                                    (("cq", "sq"), ("ck", "sk"))):
                                hs = slice(half * 512, (half + 1) * 512)
                                z3 = zraw[:, t, hs].rearrange("p (h d) -> p h d", d=D)
                                r3 = rots[t][:, hs].rearrange("p (h d) -> p h d", d=D)
                                t3 = tmps[t][:, hs].rearrange("p (h d) -> p h d", d=D)
                                cb = cos_sin[cn][:, t, :].unsqueeze(1)\
                                    .broadcast_to([P, HL, D])
                                sb = cos_sin[sn][:, t, :].unsqueeze(1)\
                                    .broadcast_to([P, HL, D])
                                nc.vector.tensor_mul(out=t3, in0=z3, in1=cb)
                                nc.vector.tensor_mul(out=r3, in0=r3, in1=sb)
                                nc.gpsimd.tensor_tensor(
                                    out=z3, in0=t3, in1=r3, op=mybir.AluOpType.add)
                        for t in ts:
                            nsl = slice(t * P, (t + 1) * P)
                            for pr in range(8):
                                tp = psA.tile([P, P], F32, tag="tp", name=f"tp{t}_{pr}")
                                nc.tensor.transpose(
                                    tp, zraw[:, t, pr * P:(pr + 1) * P], idf_sb)
                                nc.scalar.copy(out=qkt_sb[:, pr, nsl], in_=tp)

                    # ladder pipeline: stats/A3 follow A1 at 2-tile grain
                    for t in range(2):
                        a1_tile(t)
                    if do_ln:
                        stats_pair(0)
                    for g in range(1, 4):
                        a1_tile(2 * g)
                        a1_tile(2 * g + 1)
                        if do_ln:
                            stats_pair(g)
                            a3_tiles(range(2 * (g - 1), 2 * g))
                    if do_ln:
                        a3_tiles(range(6, NT))

                if not do_ln or 'B' not in phases:
                    return
                # ============ phase B+C, stage-major within each query block
                with (
                    tc.tile_pool(name="pB", bufs=2) as pB,
                    tc.tile_pool(name="psB", bufs=2, space="PSUM") as psB,
                ):
                    mk3 = mk_sb.rearrange("p (b j) -> p b j", j=P)
                    digits = [int(ch) for ch in phases if ch.isdigit()]
                    bmax = digits[0] if digits else 9
                    for qi in range(NT):
                        qsl = slice(qi * P, (qi + 1) * P)
                        kb0 = max(qi - 2, 0)
                        nkb = qi - kb0 + 1
                        NK = nkb * P
                        den8 = pB.tile([P, HL], F32, tag="den8")
                        rec8 = pB.tile([P, HL], F32, tag="rec8")
                        p_ts, scs, ptss, ats = [], [], [], []
                        # scores (PE)
                        for h in range(HL):
                            pair, poff = h // 2, (h % 2) * 64
                            sc = psB.tile([P, 3 * P], F32, tag=f"sc{h % 2}", bufs=2, name=f"sc{h}")
                            nc.tensor.matmul(
                                sc[:, 0:NK],
                                qkt_sb[poff:poff + 64, pair, qsl],
                                qkt_sb[poff:poff + 64, 4 + pair,
                                       kb0 * P:kb0 * P + NK],
                                start=True, stop=True)
                            scs.append(sc)
                        # mask add (DVE, on psum) then exp (ACT, accum->den)
                        for h in range(HL):
                            sc = scs[h]
                            if qi == 0:
                                nc.vector.tensor_add(
                                    out=sc[:, 0:P], in0=sc[:, 0:P],
                                    in1=mk_sb[:, P:2 * P])
                            elif qi == 1:
                                nc.vector.tensor_add(
                                    out=sc[:, P:2 * P], in0=sc[:, P:2 * P],
                                    in1=mk_sb[:, P:2 * P])
                            else:
                                scv = bass.AP(
                                    tensor=sc.tensor, offset=sc.offset,
                                    ap=[sc.ap[0], [2 * P, 2], [1, P]])
                                nc.vector.tensor_add(out=scv, in0=scv, in1=mk3)
                        for h in range(HL):
                            p_t = pB.tile([P, 3 * P], BF16, tag=f"p{h}", bufs=3, name=f"pt{h}")
                            nc.scalar.activation(
                                out=p_t[:, 0:NK], in_=scs[h][:, 0:NK],
                                func=mybir.ActivationFunctionType.Exp, scale=0.125,
                                accum_out=den8[:, h:h + 1])
                            p_ts.append(p_t)
                        # den comes free from exp accum_out; per-head recip
                        for h in range(0 if bmax < 2 else HL):
                            hh = slice(h, h + 1)
                            nc.vector.tensor_add(
                                out=den8[:, hh], in0=den8[:, hh],
                                in1=es_sb[:, hh])
                            nc.vector.reciprocal(
                                out=rec8[:, hh], in_=den8[:, hh])
                        # scale (DVE) + transpose (PE) + evict (ACT/DVE)
                        for h in range(0 if bmax < 3 else HL):
                            p_t = p_ts[h]
                            nc.vector.tensor_scalar_mul(
                                out=p_t[:, 0:NK], in0=p_t[:, 0:NK],
                                scalar1=rec8[:, h:h + 1])
                            ptp = psB.tile([P, 3, P], BF16, tag=f"ptp{h % 2}", bufs=1, name=f"ptp{h}")
                            for j in range(nkb):
                                nc.tensor.transpose(
                                    ptp[:, j, :], p_t[:, j * P:(j + 1) * P], idb_sb)
                            pts = pB.tile([P, 3, P], BF16, tag=f"pts{h % 2}", bufs=2, name=f"pts{h}")
                            if h % 2 == 0:
                                nc.scalar.copy(out=pts[:, 0:nkb, :],
                                               in_=ptp[:, 0:nkb, :])
                            else:
                                nc.vector.tensor_copy(out=pts[:, 0:nkb, :],
                                                      in_=ptp[:, 0:nkb, :])
                            ptss.append(pts)
                        # PV (PE) + attn evict (ACT)
                        at = None
                        for h in range(0 if bmax < 4 else HL):
                            pair, poff = h // 2, (h % 2) * 64
                            if h % 2 == 0:
                                at = psB.tile([P, P], F32, tag="at", bufs=1)
                            for j in range(nkb):
                                kb = kb0 + j
                                nc.tensor.matmul(
                                    at[poff:poff + 64, :],
                                    v_sb[:, kb, h * D:(h + 1) * D],
                                    ptss[h][:, j, :],
                                    start=(j == 0), stop=(j == nkb - 1))
                            if h % 2 == 1:
                                if pair % 2 == 0:
                                    nc.scalar.copy(
                                        out=att_sb[:, pair, qi, :], in_=at)
                                else:
                                    nc.vector.tensor_copy(
                                        out=att_sb[:, pair, qi, :], in_=at)
                        # ============ phase C: proj
                        for e in range(2 if 'C' in phases else 0):
                            pj_ps = psB.tile([P, 512], F32, tag="pjp", bufs=1)
                            for ch in range(4):
                                nc.tensor.matmul(
                                    pj_ps,
                                    att_sb[:, ch, qi, :],
                                    pj_sb[:, ch, e * 512:(e + 1) * 512],
                                    start=(ch == 0), stop=(ch == 3))
                            y_sb = pB.tile([P, 512], F32, tag="ysb")
                            nc.vector.tensor_copy(out=y_sb, in_=pj_ps)
                            nc.sync.dma_start(
                                out=y[qsl, e * 512:(e + 1) * 512], in_=y_sb)

            if use_for_i and repeat > 1:
                with tc.For_i(0, repeat, 1):
                    body()
            else:
                for _ in range(repeat):
                    body()

    nc.finalize()
    return nc


def host_prep(x, qkv_w, qn_w, qn_b, kn_w, kn_b, sinks, proj_w, proj_b):
    """Build the 8 per-core input maps (numpy, host-side sharding + tables)."""
    f32 = np.float32
    n = np.arange(N, dtype=np.float64)
    inv = ROPE_BASE ** (-np.arange(0, D, 2, dtype=np.float64) / D)
    freqs = n[:, None] * inv[None, :]
    emb = np.concatenate([freqs, freqs], axis=1)
    cos, sin = np.cos(emb), np.sin(emb)
    sgn = np.concatenate([-np.ones(D // 2), np.ones(D // 2)])

    def tables(w):
        w = np.asarray(w, np.float64)
        w_rot = np.concatenate([w[D // 2:], w[:D // 2]])
        cw = (cos * w[None, :]).astype(f32)
        sw = (sin * w_rot[None, :] * sgn[None, :]).astype(f32)
        return np.ascontiguousarray(cw), np.ascontiguousarray(sw)

    coswq, sinwq = tables(qn_w)
    coswk, sinwk = tables(kn_w)
    assert np.allclose(qn_b, 0) and np.allclose(kn_b, 0), \
        "nonzero qk-norm bias not implemented"

    r = np.arange(P)[:, None]
    c = np.arange(P)[None, :]
    m_up = np.where(c > r, 0.0, NEG)
    m_lo = np.where(c <= r, 0.0, NEG)
    masks_np = np.ascontiguousarray(
        np.concatenate([m_up, m_lo], axis=1).astype(ml_dtypes.bfloat16))
    identf_np = np.eye(P, dtype=f32)
    identb_np = np.eye(P).astype(ml_dtypes.bfloat16)

    in_maps = []
    for core in range(8):
        b, g = core // 2, core % 2
        q_rows = qkv_w[g * 512:(g + 1) * 512]
        k_rows = qkv_w[1024 + g * 512:1024 + (g + 1) * 512]
        v_rows = qkv_w[2048 + g * 512:2048 + (g + 1) * 512]
        in_maps.append({
            "xt": np.ascontiguousarray(x[b].T.astype(ml_dtypes.bfloat16)),
            "wqkt": np.ascontiguousarray(
                np.concatenate([q_rows, k_rows], 0).T.astype(ml_dtypes.bfloat16)),
            "wvt": np.ascontiguousarray(v_rows.T.astype(ml_dtypes.bfloat16)),
            "projt": np.ascontiguousarray(
                proj_w[:, g * 512:(g + 1) * 512].T.astype(f32)),
            "coswq": coswq, "sinwq": sinwq,
            "coswk": coswk, "sinwk": sinwk,
            "esink": np.exp(sinks[g * 8:(g + 1) * 8]).astype(f32).reshape(1, HL),
            "masks01": masks_np,
            "identf": identf_np,
            "identb": identb_np,
        })
    return in_maps


def assemble(results, proj_b):
    out = np.zeros((B, N, DIM), dtype=np.float32)
    for b in range(B):
        out[b] = results[2 * b]["y"] + results[2 * b + 1]["y"] + proj_b[None, :]
    return out


# ---------------------------------------------------------------------------
# Public entry point: kernel(**inputs) -> full output [B, N, DIM]
# ---------------------------------------------------------------------------
from concourse.bass_utils import run_bass_kernel_spmd

_NC_CACHE = {}


def _get_nc():
    if "nc" not in _NC_CACHE:
        _NC_CACHE["nc"] = build_nc(repeat=1)
    return _NC_CACHE["nc"]


def kernel(x, qkv_w, qn_w, qn_b, kn_w, kn_b, sinks, proj_w, proj_b):
    x = np.asarray(x, np.float32)
    qkv_w = np.asarray(qkv_w, np.float32)
    proj_w = np.asarray(proj_w, np.float32)
    in_maps = host_prep(x, qkv_w, np.asarray(qn_w), np.asarray(qn_b),
                        np.asarray(kn_w), np.asarray(kn_b),
                        np.asarray(sinks), proj_w, np.asarray(proj_b))
    nc = _get_nc()
    res = run_bass_kernel_spmd(nc, in_maps, core_ids=list(range(8)))
    return assemble(res.results, np.asarray(proj_b, np.float32))

